# revision 17
# baseline (speedup 1.0000x reference)
"""CPC spatial BCE loss kernel for 8 TRN2 NeuronCores.

Computation: loss = BCE(sigmoid((V1.reshape(N,D) @ V2.reshape(N,D).T) / D), eye(N))
with N=256, D=64*64*64=262144.

Strategy (memory-regime): shard the contraction dim D across the 8 cores
(64 MB of fp32 input per core -- the minimal possible HBM traffic). Each
core computes a partial Gram matrix [256, 256] over its 32768-wide slice
of D via bf16 TensorE matmuls accumulated in fp32 PSUM. The host lays out
each core's chunk d-major and pre-tiled to the exact SBUF tile layout, so
every chunk DMA is one fully-contiguous read with the contraction dim
landing on SBUF partitions -- no on-device transposes.

Device pipeline per chunk: fp32 loads stream on the two HWDGE rings (SP
ring for the f1 tile, ACT ring for the adjacent f2 tile of the same
packed buffer -- together they stream at ~345-428 GB/s, the per-core HBM
share to fabric ceiling); DVE casts fp32->bf16 (fp32 matmul on trn2
costs 4x: two HI/LO passes at half streaming rate, so bf16 compute is
required to stay under the DMA; SP/ACT stay compute-free so their rings
never stall); TensorE runs the matmuls into 2 PSUM accumulators. Chunk
sizes are small at the head (fast pipeline fill) and tail (short drain
after the last DMA).

The partial Gram matrices are summed on the host (the unshard step for a
sum-sharded value) and the final sigmoid+BCE over 256x256 values is a
negligible epilogue done in numpy.
"""

import numpy as np

N = 256
D = 64 * 64 * 64  # 262144
NCORES = 8
DLOC = D // NCORES  # 32768
P = 128  # SBUF partitions
MB = 16  # max d-blocks of 128 per DMA chunk

_built = {}
_last_results = None  # test harness reads profiling info from here


def _sched(dloc=DLOC, mb=MB):
    """Chunk schedule in 128-d blocks: small head (fill) / tail (drain)."""
    nblocks = dloc // P
    if nblocks <= 4 * mb:
        sched = []
        rem = nblocks
        while rem > 0:
            s = min(4, rem)
            sched.append(s)
            rem -= s
        return sched
    sched = [2, 2, 4, 8]
    while sum(sched) + mb <= nblocks - 8:
        sched.append(mb)
    rem = nblocks - sum(sched)
    while rem > 6:
        sched.append(4)
        rem -= 4
    while rem > 0:
        s = min(2, rem)
        sched.append(s)
        rem -= s
    assert sum(sched) == nblocks, (sched, nblocks)
    return sched


def _stripe_plan(dloc, mb, stripe):
    """Assign chunk c -> (tensor c%stripe, row offset within it)."""
    sched = _sched(dloc, mb)
    offs, sizes = [], [0] * stripe
    for c, cmb in enumerate(sched):
        t = c % stripe
        offs.append((t, sizes[t]))
        sizes[t] += 2 * cmb * P
    return sched, offs, sizes


def _build(dloc=DLOC, mb=MB, compute_dtype="fp8", bufs=3, bufs_f=None,
           bufs_b=None, stripe=1, nwarm=10, nfill=3, fill_until=12):
    """Build + bacc-compile the per-core Bass kernel.

    Per-core inputs f1t, f2t: [dloc, N] fp32, host pre-tiled so chunk c
    (covering blocks [b0, b0+cmb) of 128 d-values) occupies rows
    [b0*P, (b0+cmb)*P) with row r = b0*P + p*cmb + nb holding
    f.T[d, :] for d = core_off + (b0+nb)*P + p  -- i.e. each chunk DMA
    is one contiguous read mapping partition p <- d within block.
    Output: out[i, j] = sum_d f1t[d, i] * f2t[d, j]   (partial Gram)
    """
    import concourse.mybir as mybir
    from concourse import bacc
    from concourse.bass import MemorySpace
    from concourse.tile import TileContext

    nblocks = dloc // P

    # fp8 mode: inputs are cast to fp8e4 on the host, so HBM traffic is
    # 1/4 of fp32 and there is no on-device cast at all. TensorE runs
    # DoubleRow matmuls (2 k-blocks of 128 per instruction, 2 fp8 values
    # per PE cell) to stay under the DMA cadence.
    if compute_dtype == "fp8":
        idt = cdt = mybir.dt.float8e4
    elif compute_dtype == "bf16":
        idt, cdt = mybir.dt.float32, mybir.dt.bfloat16
    else:
        idt = cdt = mybir.dt.float32

    nc = bacc.Bacc("TRN2", target_bir_lowering=False, debug=False,
                   num_devices=NCORES)
    # Both tensors packed chunk-interleaved [f1_c | f2_c | f1_c+1 ...] so the
    # two concurrent ring reads hit adjacent address regions (uniform HBM
    # channel striping instead of two far-apart colliding streams). With
    # stripe > 1, chunks round-robin over separate DRAM allocations.
    sched, offs, sizes = _stripe_plan(dloc, mb, stripe)
    fts = [nc.dram_tensor(f"ft{s}", (sizes[s], N), idt,
                          kind="ExternalInput")
           for s in range(stripe)]
    out = nc.dram_tensor("out", (N, N), mybir.dt.float32,
                         kind="ExternalOutput")

    fvs = [t.ap() for t in fts]

    with TileContext(nc) as tc:
        with tc.tile_pool(name="psum", bufs=1, space=MemorySpace.PSUM) as psum_pool, \
             tc.tile_pool(name="sbuff", bufs=bufs_f or bufs + 1) as poolf, \
             tc.tile_pool(name="sbufb", bufs=bufs_b or bufs) as poolb, \
             tc.tile_pool(name="outp", bufs=1) as outpool:
            acc = [psum_pool.tile([P, N], mybir.dt.float32, tag=f"acc{ib}",
                                  name=f"acc{ib}")
                   for ib in range(2)]
            if nwarm:
                # PE warm-up: the HAM clock gate keeps the PE at 1.2 GHz
                # until it sees ~3.4us of sustained activity. Without this,
                # the first ~14us of real matmuls run at half clock, the
                # cold PE outpaces nothing and the DMA rings throttle on
                # full SBUF buffers. These dummy matmuls run during the
                # first chunk DMAs (they depend only on the memset) and
                # flip the gate before real data lands.
                wsrc = outpool.tile([P, 512], cdt, tag="wsrc", name="wsrc")
                wacc = psum_pool.tile([P, 512], mybir.dt.float32,
                                      tag="wacc", name="wacc")
                nc.any.memset(wsrc, 0)
                for i in range(nwarm):
                    nc.tensor.matmul(wacc, wsrc[:, :P], wsrc,
                                     start=True, stop=True)

                def _keep_warm(n):
                    # HAM watches PE duty cycle per free-running ~3.4us
                    # window; the DMA-paced gaps between chunk bursts
                    # during pipeline fill re-throttle the clock (and a
                    # cold PE then paces the pipeline). Dummy matmuls
                    # absorb the idle gap.
                    for _ in range(n):
                        nc.tensor.matmul(wacc, wsrc[:, :P], wsrc,
                                         start=True, stop=True)
            else:
                def _keep_warm(n):
                    pass
            b0 = 0
            for c, cmb in enumerate(sched):
                # f1 tile and f2 tile are adjacent in the packed buffer;
                # the two HWDGE rings (SP / ACT) stream them concurrently.
                tgt, r0 = offs[c]
                fv = fvs[tgt]
                r1 = fv[r0:r0 + cmb * P]
                r2 = fv[r0 + cmb * P:r0 + 2 * cmb * P]
                tf = poolf.tile([P, 2, mb, N], idt, tag="tf",
                                name=f"tf_{c}")[:, :, :cmb]
                nc.sync.dma_start(
                    out=tf[:, 0], in_=r1.rearrange("(p nb) i -> p nb i", p=P))
                nc.scalar.dma_start(
                    out=tf[:, 1], in_=r2.rearrange("(p nb) i -> p nb i", p=P))
                if cdt == idt:
                    tb = tf
                else:
                    # cast fp32->bf16 on DVE only: SP/ACT stay pure DMA rings
                    # (casts on ACT block its ring's next DMA issue in the
                    # engine FIFO; GpSimd shares its SBUF port with DVE and
                    # just contends). DVE ~0.6ns/elem stays under the chunk
                    # DMA cadence.
                    tb = poolb.tile([P, 2, mb, N], cdt, tag="tb",
                                    name=f"tb_{c}")[:, :, :cmb]
                    nc.vector.tensor_copy(tb, tf)
                last_chunk = c == len(sched) - 1
                if cdt == mybir.dt.float8e4:
                    # DoubleRow: one matmul contracts 2 k-blocks (256 deep),
                    # 2 fp8 elements per PE cell. lhsT/rhs get 3D APs
                    # [K=128, 2, cols]; the (k, pair) -> d mapping just has
                    # to agree between the two operands, which it does since
                    # both tiles share the same [P, nb, N] layout.
                    assert cmb % 2 == 0, cmb
                    dr = mybir.MatmulPerfMode.DoubleRow
                    if not last_chunk:
                        for nb in range(0, cmb, 2):
                            gb = b0 + nb
                            for ib in range(2):
                                nc.tensor.matmul(
                                    acc[ib],
                                    tb[:, 0, nb:nb + 2, ib * P:(ib + 1) * P],
                                    tb[:, 1, nb:nb + 2, :],
                                    start=(gb == 0),
                                    stop=False,
                                    perf_mode=dr,
                                )
                        if c < fill_until:
                            _keep_warm(nfill)
                    else:
                        for ib in range(2):
                            for nb in range(0, cmb, 2):
                                nc.tensor.matmul(
                                    acc[ib],
                                    tb[:, 0, nb:nb + 2, ib * P:(ib + 1) * P],
                                    tb[:, 1, nb:nb + 2, :],
                                    start=False,
                                    stop=(nb == cmb - 2),
                                    perf_mode=dr,
                                )
                            o = outpool.tile([P, N], mybir.dt.float32,
                                             tag=f"o{ib}", name=f"o{ib}")
                            # different engines/rings per half so the two
                            # copies + stores drain in parallel
                            if ib == 0:
                                nc.vector.tensor_copy(o, acc[ib])
                                nc.sync.dma_start(
                                    out=out.ap()[ib * P:(ib + 1) * P, :],
                                    in_=o)
                            else:
                                nc.scalar.copy(o, acc[ib])
                                nc.scalar.dma_start(
                                    out=out.ap()[ib * P:(ib + 1) * P, :],
                                    in_=o)
                elif not last_chunk:
                    for nb in range(cmb):
                        gb = b0 + nb
                        for ib in range(2):
                            nc.tensor.matmul(
                                acc[ib],
                                tb[:, 0, nb, ib * P:(ib + 1) * P],  # lhsT
                                tb[:, 1, nb, :],                    # rhs
                                start=(gb == 0),
                                stop=False,
                            )
                else:
                    # ib-major in the last chunk: acc[0] finishes first so
                    # its PSUM copy + store overlap acc[1]'s final matmuls.
                    for ib in range(2):
                        for nb in range(cmb):
                            nc.tensor.matmul(
                                acc[ib],
                                tb[:, 0, nb, ib * P:(ib + 1) * P],
                                tb[:, 1, nb, :],
                                start=False,
                                stop=(nb == cmb - 1),
                            )
                        o = outpool.tile([P, N], mybir.dt.float32,
                                         tag=f"o{ib}", name=f"o{ib}")
                        nc.vector.tensor_copy(o, acc[ib])
                        nc.sync.dma_start(
                            out=out.ap()[ib * P:(ib + 1) * P, :], in_=o)
                b0 += cmb

    nc.compile()
    return nc


def _get_nc():
    if "nc" not in _built:
        _built["nc"] = _build(bufs_f=8)
    return _built["nc"]


def _gram_partials(in_maps, trace=False):
    global _last_results
    from concourse.bass_utils import run_bass_kernel_spmd

    nc = _get_nc()
    res = run_bass_kernel_spmd(nc, in_maps, core_ids=list(range(NCORES)),
                               trace=trace)
    _last_results = res
    return [r["out"] for r in res.results]


def _pack_core(f1, f2, k, dloc=DLOC, mb=MB, stripe=1, dtype=None):
    """Pack core k's d-chunks of f1, f2 [N, D] into one [2*dloc, N] buffer.

    Chunk c covering blocks [b0, b0+cmb): the f1 tile occupies rows
    [2*b0*P, (2*b0+cmb)*P) and the f2 tile the next cmb*P rows, each in
    [P, cmb, N] SBUF tile order (row p*cmb+nb holds f[:, (b0+nb)*P+p]).

    dtype: on-wire dtype (default fp8e4). Cast happens before the
    transpose so the shuffle moves 1-byte elements.
    """
    if dtype is None:
        import ml_dtypes
        dtype = ml_dtypes.float8_e4m3
    x1 = f1[:, k * dloc:(k + 1) * dloc].astype(dtype)
    x2 = f2[:, k * dloc:(k + 1) * dloc].astype(dtype)
    sched, offs, sizes = _stripe_plan(dloc, mb, stripe)
    outs = [np.empty((sz, N), dtype=dtype) for sz in sizes]
    b0 = 0
    for c, cmb in enumerate(sched):
        tgt, r0 = offs[c]
        for j, x in enumerate((x1, x2)):
            sl = x[:, b0 * P:(b0 + cmb) * P]              # [N, cmb*P]
            t = sl.reshape(N, cmb, P).transpose(2, 1, 0)  # [P, cmb, N]
            rr = r0 + j * cmb * P
            outs[tgt][rr:rr + cmb * P] = t.reshape(cmb * P, N)
        b0 += cmb
    return {f"ft{s}": outs[s] for s in range(stripe)}


def kernel(V1, V2):
    V1 = np.asarray(V1, dtype=np.float32)
    V2 = np.asarray(V2, dtype=np.float32)
    f1 = V1.reshape(N, D)
    f2 = V2.reshape(N, D)

    from concurrent.futures import ThreadPoolExecutor
    with ThreadPoolExecutor(NCORES) as ex:
        in_maps = list(ex.map(lambda k: _pack_core(f1, f2, k), range(NCORES)))
    partials = _gram_partials(in_maps)

    Z = np.zeros((N, N), dtype=np.float64)
    for pmat in partials:
        Z += pmat
    Z /= D

    eps = 1e-12
    p = 1.0 / (1.0 + np.exp(-Z))
    p = np.clip(p, eps, 1.0 - eps)
    lab = np.eye(N, dtype=np.float64)
    loss = -np.mean(lab * np.log(p) + (1.0 - lab) * np.log1p(-p))
    return np.array(loss, dtype=np.float32)


def _selftest_sim():
    """Scaled-down correctness check in CoreSim (no hardware)."""
    import ml_dtypes
    from concourse.bass_interp import CoreSim

    dloc, mb = 2048, 4
    nc = _build(dloc=dloc, mb=mb)
    rng = np.random.default_rng(0)
    a = rng.standard_normal((N, dloc)).astype(np.float32)  # [N, dloc] like f1
    b = rng.standard_normal((N, dloc)).astype(np.float32)

    sim = CoreSim(nc)
    for name, arr in _pack_core(a, b, 0, dloc=dloc, mb=mb).items():
        sim.tensor(name)[:] = arr
    sim.simulate()
    got = np.array(sim.tensor("out"))
    # expected: exact product of the fp8-quantized inputs
    qa = a.astype(ml_dtypes.float8_e4m3).astype(np.float64)
    qb = b.astype(ml_dtypes.float8_e4m3).astype(np.float64)
    want = qa @ qb.T
    err = np.abs(got - want).max() / np.abs(want).max()
    print("selftest rel err vs quantized product:", err)
    assert err < 1e-2, err
    print("SELFTEST PASSED")


if __name__ == "__main__":
    _selftest_sim()



# revision 19
# speedup vs baseline: 1.0446x; 1.0446x over previous
"""CPC spatial BCE loss kernel for 8 TRN2 NeuronCores.

Computation: loss = BCE(sigmoid((V1.reshape(N,D) @ V2.reshape(N,D).T) / D), eye(N))
with N=256, D=64*64*64=262144.

Strategy (memory-regime): shard the contraction dim D across the 8 cores
(64 MB of fp32 input per core -- the minimal possible HBM traffic). Each
core computes a partial Gram matrix [256, 256] over its 32768-wide slice
of D via bf16 TensorE matmuls accumulated in fp32 PSUM. The host lays out
each core's chunk d-major and pre-tiled to the exact SBUF tile layout, so
every chunk DMA is one fully-contiguous read with the contraction dim
landing on SBUF partitions -- no on-device transposes.

Device pipeline per chunk: fp32 loads stream on the two HWDGE rings (SP
ring for the f1 tile, ACT ring for the adjacent f2 tile of the same
packed buffer -- together they stream at ~345-428 GB/s, the per-core HBM
share to fabric ceiling); DVE casts fp32->bf16 (fp32 matmul on trn2
costs 4x: two HI/LO passes at half streaming rate, so bf16 compute is
required to stay under the DMA; SP/ACT stay compute-free so their rings
never stall); TensorE runs the matmuls into 2 PSUM accumulators. Chunk
sizes are small at the head (fast pipeline fill) and tail (short drain
after the last DMA).

The partial Gram matrices are summed on the host (the unshard step for a
sum-sharded value) and the final sigmoid+BCE over 256x256 values is a
negligible epilogue done in numpy.
"""

import numpy as np

N = 256
D = 64 * 64 * 64  # 262144
NCORES = 8
DLOC = D // NCORES  # 32768
P = 128  # SBUF partitions
MB = 16  # max d-blocks of 128 per DMA chunk

_built = {}
_last_results = None  # test harness reads profiling info from here


def _sched(dloc=DLOC, mb=MB):
    """Chunk schedule in 128-d blocks: small head (fill) / tail (drain)."""
    nblocks = dloc // P
    if nblocks <= 4 * mb:
        sched = []
        rem = nblocks
        while rem > 0:
            s = min(4, rem)
            sched.append(s)
            rem -= s
        return sched
    # Head [8,8]: big enough for efficient DMA descriptor lines (2KB per
    # partition), small enough to start compute early. Body: mb-block
    # chunks. Tail [*, 2]: the drain after the last DMA byte is just one
    # 2-block chunk; per-chunk sem/restart overhead made a long small
    # tail a net loss.
    sched = [8, 8]
    while sum(sched) + mb <= nblocks - 16:
        sched.append(mb)
    rem = nblocks - sum(sched)
    if rem > 2:
        sched.append(rem - 2)
        sched.append(2)
    elif rem:
        sched.append(rem)
    assert sum(sched) == nblocks, (sched, nblocks)
    assert all(s % 2 == 0 for s in sched), sched
    return sched


def _stripe_plan(dloc, mb, stripe):
    """Assign chunk c -> (tensor c%stripe, row offset within it)."""
    sched = _sched(dloc, mb)
    offs, sizes = [], [0] * stripe
    for c, cmb in enumerate(sched):
        t = c % stripe
        offs.append((t, sizes[t]))
        sizes[t] += 2 * cmb * P
    return sched, offs, sizes


def _build(dloc=DLOC, mb=MB, compute_dtype="fp8", bufs=3, bufs_f=None,
           bufs_b=None, stripe=1, nwarm=10, nfill=0, fill_until=12):
    """Build + bacc-compile the per-core Bass kernel.

    Per-core inputs f1t, f2t: [dloc, N] fp32, host pre-tiled so chunk c
    (covering blocks [b0, b0+cmb) of 128 d-values) occupies rows
    [b0*P, (b0+cmb)*P) with row r = b0*P + p*cmb + nb holding
    f.T[d, :] for d = core_off + (b0+nb)*P + p  -- i.e. each chunk DMA
    is one contiguous read mapping partition p <- d within block.
    Output: out[i, j] = sum_d f1t[d, i] * f2t[d, j]   (partial Gram)
    """
    import concourse.mybir as mybir
    from concourse import bacc
    from concourse.bass import MemorySpace
    from concourse.tile import TileContext

    nblocks = dloc // P

    # fp8 mode: inputs are cast to fp8e4 on the host, so HBM traffic is
    # 1/4 of fp32 and there is no on-device cast at all. TensorE runs
    # DoubleRow matmuls (2 k-blocks of 128 per instruction, 2 fp8 values
    # per PE cell) to stay under the DMA cadence.
    if compute_dtype == "fp8":
        idt = cdt = mybir.dt.float8e4
    elif compute_dtype == "bf16":
        idt, cdt = mybir.dt.float32, mybir.dt.bfloat16
    else:
        idt = cdt = mybir.dt.float32

    nc = bacc.Bacc("TRN2", target_bir_lowering=False, debug=False,
                   num_devices=NCORES)
    # Both tensors packed chunk-interleaved [f1_c | f2_c | f1_c+1 ...] so the
    # two concurrent ring reads hit adjacent address regions (uniform HBM
    # channel striping instead of two far-apart colliding streams). With
    # stripe > 1, chunks round-robin over separate DRAM allocations.
    sched, offs, sizes = _stripe_plan(dloc, mb, stripe)
    fts = [nc.dram_tensor(f"ft{s}", (sizes[s], N), idt,
                          kind="ExternalInput")
           for s in range(stripe)]
    out = nc.dram_tensor("out", (N, N), mybir.dt.float32,
                         kind="ExternalOutput")

    fvs = [t.ap() for t in fts]

    with TileContext(nc) as tc:
        with tc.tile_pool(name="psum", bufs=1, space=MemorySpace.PSUM) as psum_pool, \
             tc.tile_pool(name="sbuff", bufs=bufs_f or bufs + 1) as poolf, \
             tc.tile_pool(name="sbufb", bufs=bufs_b or bufs) as poolb, \
             tc.tile_pool(name="outp", bufs=1) as outpool:
            acc = [psum_pool.tile([P, N], mybir.dt.float32, tag=f"acc{ib}",
                                  name=f"acc{ib}")
                   for ib in range(2)]
            if nwarm:
                # PE warm-up: the HAM clock gate keeps the PE at 1.2 GHz
                # until it sees ~3.4us of sustained activity. Without this,
                # the first ~14us of real matmuls run at half clock, the
                # cold PE outpaces nothing and the DMA rings throttle on
                # full SBUF buffers. These dummy matmuls run during the
                # first chunk DMAs (they depend only on the memset) and
                # flip the gate before real data lands.
                wsrc = outpool.tile([P, 512], cdt, tag="wsrc", name="wsrc")
                wacc = psum_pool.tile([P, 512], mybir.dt.float32,
                                      tag="wacc", name="wacc")
                nc.any.memset(wsrc, 0)
                for i in range(nwarm):
                    nc.tensor.matmul(wacc, wsrc[:, :P], wsrc,
                                     start=True, stop=True)

                def _keep_warm(n):
                    # HAM watches PE duty cycle per free-running ~3.4us
                    # window; the DMA-paced gaps between chunk bursts
                    # during pipeline fill re-throttle the clock (and a
                    # cold PE then paces the pipeline). Dummy matmuls
                    # absorb the idle gap.
                    for _ in range(n):
                        nc.tensor.matmul(wacc, wsrc[:, :P], wsrc,
                                         start=True, stop=True)
            else:
                def _keep_warm(n):
                    pass
            b0 = 0
            for c, cmb in enumerate(sched):
                # f1 tile and f2 tile are adjacent in the packed buffer;
                # the two HWDGE rings (SP / ACT) stream them concurrently.
                tgt, r0 = offs[c]
                fv = fvs[tgt]
                r1 = fv[r0:r0 + cmb * P]
                r2 = fv[r0 + cmb * P:r0 + 2 * cmb * P]
                tf = poolf.tile([P, 2, mb, N], idt, tag="tf",
                                name=f"tf_{c}")[:, :, :cmb]
                nc.sync.dma_start(
                    out=tf[:, 0], in_=r1.rearrange("(p nb) i -> p nb i", p=P))
                nc.scalar.dma_start(
                    out=tf[:, 1], in_=r2.rearrange("(p nb) i -> p nb i", p=P))
                if cdt == idt:
                    tb = tf
                else:
                    # cast fp32->bf16 on DVE only: SP/ACT stay pure DMA rings
                    # (casts on ACT block its ring's next DMA issue in the
                    # engine FIFO; GpSimd shares its SBUF port with DVE and
                    # just contends). DVE ~0.6ns/elem stays under the chunk
                    # DMA cadence.
                    tb = poolb.tile([P, 2, mb, N], cdt, tag="tb",
                                    name=f"tb_{c}")[:, :, :cmb]
                    nc.vector.tensor_copy(tb, tf)
                last_chunk = c == len(sched) - 1
                if cdt == mybir.dt.float8e4:
                    # DoubleRow: one matmul contracts 2 k-blocks (256 deep),
                    # 2 fp8 elements per PE cell. lhsT/rhs get 3D APs
                    # [K=128, 2, cols]; the (k, pair) -> d mapping just has
                    # to agree between the two operands, which it does since
                    # both tiles share the same [P, nb, N] layout.
                    assert cmb % 2 == 0, cmb
                    dr = mybir.MatmulPerfMode.DoubleRow
                    if not last_chunk:
                        for nb in range(0, cmb, 2):
                            gb = b0 + nb
                            for ib in range(2):
                                nc.tensor.matmul(
                                    acc[ib],
                                    tb[:, 0, nb:nb + 2, ib * P:(ib + 1) * P],
                                    tb[:, 1, nb:nb + 2, :],
                                    start=(gb == 0),
                                    stop=False,
                                    perf_mode=dr,
                                )
                        if c < fill_until:
                            _keep_warm(nfill)
                    else:
                        for ib in range(2):
                            for nb in range(0, cmb, 2):
                                nc.tensor.matmul(
                                    acc[ib],
                                    tb[:, 0, nb:nb + 2, ib * P:(ib + 1) * P],
                                    tb[:, 1, nb:nb + 2, :],
                                    start=False,
                                    stop=(nb == cmb - 2),
                                    perf_mode=dr,
                                )
                            o = outpool.tile([P, N], mybir.dt.float32,
                                             tag=f"o{ib}", name=f"o{ib}")
                            # different engines/rings per half so the two
                            # copies + stores drain in parallel
                            if ib == 0:
                                nc.vector.tensor_copy(o, acc[ib])
                                nc.sync.dma_start(
                                    out=out.ap()[ib * P:(ib + 1) * P, :],
                                    in_=o)
                            else:
                                nc.scalar.copy(o, acc[ib])
                                nc.scalar.dma_start(
                                    out=out.ap()[ib * P:(ib + 1) * P, :],
                                    in_=o)
                elif not last_chunk:
                    for nb in range(cmb):
                        gb = b0 + nb
                        for ib in range(2):
                            nc.tensor.matmul(
                                acc[ib],
                                tb[:, 0, nb, ib * P:(ib + 1) * P],  # lhsT
                                tb[:, 1, nb, :],                    # rhs
                                start=(gb == 0),
                                stop=False,
                            )
                else:
                    # ib-major in the last chunk: acc[0] finishes first so
                    # its PSUM copy + store overlap acc[1]'s final matmuls.
                    for ib in range(2):
                        for nb in range(cmb):
                            nc.tensor.matmul(
                                acc[ib],
                                tb[:, 0, nb, ib * P:(ib + 1) * P],
                                tb[:, 1, nb, :],
                                start=False,
                                stop=(nb == cmb - 1),
                            )
                        o = outpool.tile([P, N], mybir.dt.float32,
                                         tag=f"o{ib}", name=f"o{ib}")
                        nc.vector.tensor_copy(o, acc[ib])
                        nc.sync.dma_start(
                            out=out.ap()[ib * P:(ib + 1) * P, :], in_=o)
                b0 += cmb

    nc.compile()
    return nc


def _get_nc():
    if "nc" not in _built:
        _built["nc"] = _build(bufs_f=8)
    return _built["nc"]


def _gram_partials(in_maps, trace=False):
    global _last_results
    from concourse.bass_utils import run_bass_kernel_spmd

    nc = _get_nc()
    res = run_bass_kernel_spmd(nc, in_maps, core_ids=list(range(NCORES)),
                               trace=trace)
    _last_results = res
    return [r["out"] for r in res.results]


def _pack_core(f1, f2, k, dloc=DLOC, mb=MB, stripe=1, dtype=None):
    """Pack core k's d-chunks of f1, f2 [N, D] into one [2*dloc, N] buffer.

    Chunk c covering blocks [b0, b0+cmb): the f1 tile occupies rows
    [2*b0*P, (2*b0+cmb)*P) and the f2 tile the next cmb*P rows, each in
    [P, cmb, N] SBUF tile order (row p*cmb+nb holds f[:, (b0+nb)*P+p]).

    dtype: on-wire dtype (default fp8e4). Cast happens before the
    transpose so the shuffle moves 1-byte elements.
    """
    if dtype is None:
        import ml_dtypes
        dtype = ml_dtypes.float8_e4m3
    x1 = f1[:, k * dloc:(k + 1) * dloc].astype(dtype)
    x2 = f2[:, k * dloc:(k + 1) * dloc].astype(dtype)
    sched, offs, sizes = _stripe_plan(dloc, mb, stripe)
    outs = [np.empty((sz, N), dtype=dtype) for sz in sizes]
    b0 = 0
    for c, cmb in enumerate(sched):
        tgt, r0 = offs[c]
        for j, x in enumerate((x1, x2)):
            sl = x[:, b0 * P:(b0 + cmb) * P]              # [N, cmb*P]
            t = sl.reshape(N, cmb, P).transpose(2, 1, 0)  # [P, cmb, N]
            rr = r0 + j * cmb * P
            outs[tgt][rr:rr + cmb * P] = t.reshape(cmb * P, N)
        b0 += cmb
    return {f"ft{s}": outs[s] for s in range(stripe)}


def kernel(V1, V2):
    V1 = np.asarray(V1, dtype=np.float32)
    V2 = np.asarray(V2, dtype=np.float32)
    f1 = V1.reshape(N, D)
    f2 = V2.reshape(N, D)

    from concurrent.futures import ThreadPoolExecutor
    with ThreadPoolExecutor(NCORES) as ex:
        in_maps = list(ex.map(lambda k: _pack_core(f1, f2, k), range(NCORES)))
    partials = _gram_partials(in_maps)

    Z = np.zeros((N, N), dtype=np.float64)
    for pmat in partials:
        Z += pmat
    Z /= D

    eps = 1e-12
    p = 1.0 / (1.0 + np.exp(-Z))
    p = np.clip(p, eps, 1.0 - eps)
    lab = np.eye(N, dtype=np.float64)
    loss = -np.mean(lab * np.log(p) + (1.0 - lab) * np.log1p(-p))
    return np.array(loss, dtype=np.float32)


def _selftest_sim():
    """Scaled-down correctness check in CoreSim (no hardware)."""
    import ml_dtypes
    from concourse.bass_interp import CoreSim

    dloc, mb = 2048, 4
    nc = _build(dloc=dloc, mb=mb)
    rng = np.random.default_rng(0)
    a = rng.standard_normal((N, dloc)).astype(np.float32)  # [N, dloc] like f1
    b = rng.standard_normal((N, dloc)).astype(np.float32)

    sim = CoreSim(nc)
    for name, arr in _pack_core(a, b, 0, dloc=dloc, mb=mb).items():
        sim.tensor(name)[:] = arr
    sim.simulate()
    got = np.array(sim.tensor("out"))
    # expected: exact product of the fp8-quantized inputs
    qa = a.astype(ml_dtypes.float8_e4m3).astype(np.float64)
    qb = b.astype(ml_dtypes.float8_e4m3).astype(np.float64)
    want = qa @ qb.T
    err = np.abs(got - want).max() / np.abs(want).max()
    print("selftest rel err vs quantized product:", err)
    assert err < 1e-2, err
    print("SELFTEST PASSED")


if __name__ == "__main__":
    _selftest_sim()



# revision 22
# speedup vs baseline: 1.0693x; 1.0237x over previous
"""CPC spatial BCE loss kernel for 8 TRN2 NeuronCores.

Computation: loss = BCE(sigmoid((V1.reshape(N,D) @ V2.reshape(N,D).T) / D), eye(N))
with N=256, D=64*64*64=262144.

Strategy (memory-regime): shard the contraction dim D across the 8 cores
and quantize to fp8e4 on the host, so each core streams 16 MB -- 1/4 of
the fp32 bytes and the minimum HBM traffic for an exact-shape Gram
computation (the 2e-2 loss tolerance dwarfs fp8 input rounding: measured
end-to-end effect ~1e-8 relative, since the loss is log(2) + O(1e-5) and
z-values are ~N(0, 1/sqrt(D))). Each core computes a partial [256, 256]
Gram over its 32768-wide slice of D. The host pre-tiles each core's
slice into the exact SBUF layout so every chunk DMA is one contiguous
read landing the contraction dim on partitions -- no on-device work but
the matmuls themselves.

Device pipeline per chunk: fp8 tiles stream on the two HWDGE rings (SP
ring f1 tile, ACT ring f2 tile; together ~410 GB/s, ~95% of the 435 GB/s
SBUF fabric ceiling); TensorE runs DoubleRow matmuls (2 fp8 values per
PE cell = 256-deep contraction per instruction) into 2 PSUM
accumulators, consuming each 1 MB chunk in ~1.75 us against the ~2.5 us
DMA cadence, so the stream stays DMA-paced. Dummy warm-up matmuls at
kernel start plus one per chunk keep the PE HAM clock gate at 2.4 GHz
(a cold 1.2 GHz PE otherwise outpaces nothing, backs up the SBUF pool,
and throttles the DMA rings mid-fill).

The partial Gram matrices are summed on the host (the unshard step for a
sum-sharded value) and the final sigmoid+BCE over 256x256 values is a
negligible epilogue done in numpy.
"""

import numpy as np

N = 256
D = 64 * 64 * 64  # 262144
NCORES = 8
DLOC = D // NCORES  # 32768
P = 128  # SBUF partitions
MB = 16  # max d-blocks of 128 per DMA chunk

_built = {}
_last_results = None  # test harness reads profiling info from here


def _sched(dloc=DLOC, mb=MB):
    """Chunk schedule in 128-d blocks: small head (fill) / tail (drain)."""
    nblocks = dloc // P
    if nblocks <= 4 * mb:
        sched = []
        rem = nblocks
        while rem > 0:
            s = min(4, rem)
            sched.append(s)
            rem -= s
        return sched
    # Head [8,8]: big enough for efficient DMA descriptor lines (2KB per
    # partition), small enough to start compute early. Body: mb-block
    # chunks. Tail [*, 2]: the drain after the last DMA byte is just one
    # 2-block chunk; per-chunk sem/restart overhead made a long small
    # tail a net loss.
    sched = [8, 8]
    while sum(sched) + mb <= nblocks - 16:
        sched.append(mb)
    rem = nblocks - sum(sched)
    if rem > 2:
        sched.append(rem - 2)
        sched.append(2)
    elif rem:
        sched.append(rem)
    assert sum(sched) == nblocks, (sched, nblocks)
    assert all(s % 2 == 0 for s in sched), sched
    return sched


def _stripe_plan(dloc, mb, stripe):
    """Assign chunk c -> (tensor c%stripe, row offset within it)."""
    sched = _sched(dloc, mb)
    offs, sizes = [], [0] * stripe
    for c, cmb in enumerate(sched):
        t = c % stripe
        offs.append((t, sizes[t]))
        sizes[t] += 2 * cmb * P
    return sched, offs, sizes


def _build(dloc=DLOC, mb=MB, compute_dtype="fp8", bufs=3, bufs_f=None,
           bufs_b=None, stripe=1, nwarm=10, nfill=0, fill_until=12):
    """Build + bacc-compile the per-core Bass kernel.

    Per-core inputs f1t, f2t: [dloc, N] fp32, host pre-tiled so chunk c
    (covering blocks [b0, b0+cmb) of 128 d-values) occupies rows
    [b0*P, (b0+cmb)*P) with row r = b0*P + p*cmb + nb holding
    f.T[d, :] for d = core_off + (b0+nb)*P + p  -- i.e. each chunk DMA
    is one contiguous read mapping partition p <- d within block.
    Output: out[i, j] = sum_d f1t[d, i] * f2t[d, j]   (partial Gram)
    """
    import concourse.mybir as mybir
    from concourse import bacc
    from concourse.bass import MemorySpace
    from concourse.tile import TileContext

    nblocks = dloc // P

    # fp8 mode: inputs are cast to fp8e4 on the host, so HBM traffic is
    # 1/4 of fp32 and there is no on-device cast at all. TensorE runs
    # DoubleRow matmuls (2 k-blocks of 128 per instruction, 2 fp8 values
    # per PE cell) to stay under the DMA cadence.
    if compute_dtype == "fp8":
        idt = cdt = mybir.dt.float8e4
    elif compute_dtype == "bf16":
        idt, cdt = mybir.dt.float32, mybir.dt.bfloat16
    else:
        idt = cdt = mybir.dt.float32

    nc = bacc.Bacc("TRN2", target_bir_lowering=False, debug=False,
                   num_devices=NCORES)
    # Both tensors packed chunk-interleaved [f1_c | f2_c | f1_c+1 ...] so the
    # two concurrent ring reads hit adjacent address regions (uniform HBM
    # channel striping instead of two far-apart colliding streams). With
    # stripe > 1, chunks round-robin over separate DRAM allocations.
    sched, offs, sizes = _stripe_plan(dloc, mb, stripe)
    fts = [nc.dram_tensor(f"ft{s}", (sizes[s], N), idt,
                          kind="ExternalInput")
           for s in range(stripe)]
    out = nc.dram_tensor("out", (N, N), mybir.dt.float32,
                         kind="ExternalOutput")

    fvs = [t.ap() for t in fts]

    with TileContext(nc) as tc:
        with tc.tile_pool(name="psum", bufs=1, space=MemorySpace.PSUM) as psum_pool, \
             tc.tile_pool(name="sbuff", bufs=bufs_f or bufs + 1) as poolf, \
             tc.tile_pool(name="sbufb", bufs=bufs_b or bufs) as poolb, \
             tc.tile_pool(name="outp", bufs=1) as outpool:
            acc = [psum_pool.tile([P, N], mybir.dt.float32, tag=f"acc{ib}",
                                  name=f"acc{ib}")
                   for ib in range(2)]
            if nwarm:
                # PE warm-up: the HAM clock gate keeps the PE at 1.2 GHz
                # until it sees ~3.4us of sustained activity. Without this,
                # the first ~14us of real matmuls run at half clock, the
                # cold PE outpaces nothing and the DMA rings throttle on
                # full SBUF buffers. These dummy matmuls run during the
                # first chunk DMAs (they depend only on the memset) and
                # flip the gate before real data lands.
                wsrc = outpool.tile([P, 512], cdt, tag="wsrc", name="wsrc")
                wacc = psum_pool.tile([P, 512], mybir.dt.float32,
                                      tag="wacc", name="wacc")
                nc.any.memset(wsrc, 0)
                for i in range(nwarm):
                    nc.tensor.matmul(wacc, wsrc[:, :P], wsrc,
                                     start=True, stop=True)

                def _keep_warm(n):
                    # HAM watches PE duty cycle per free-running ~3.4us
                    # window; the DMA-paced gaps between chunk bursts
                    # during pipeline fill re-throttle the clock (and a
                    # cold PE then paces the pipeline). Dummy matmuls
                    # absorb the idle gap.
                    for _ in range(n):
                        nc.tensor.matmul(wacc, wsrc[:, :P], wsrc,
                                         start=True, stop=True)
            else:
                def _keep_warm(n):
                    pass
            b0 = 0
            for c, cmb in enumerate(sched):
                # f1 tile and f2 tile are adjacent in the packed buffer;
                # the two HWDGE rings (SP / ACT) stream them concurrently.
                tgt, r0 = offs[c]
                fv = fvs[tgt]
                r1 = fv[r0:r0 + cmb * P]
                r2 = fv[r0 + cmb * P:r0 + 2 * cmb * P]
                tf = poolf.tile([P, 2, mb, N], idt, tag="tf",
                                name=f"tf_{c}")[:, :, :cmb]
                nc.sync.dma_start(
                    out=tf[:, 0], in_=r1.rearrange("(p nb) i -> p nb i", p=P))
                nc.scalar.dma_start(
                    out=tf[:, 1], in_=r2.rearrange("(p nb) i -> p nb i", p=P))
                if cdt == idt:
                    tb = tf
                else:
                    # cast fp32->bf16 on DVE only: SP/ACT stay pure DMA rings
                    # (casts on ACT block its ring's next DMA issue in the
                    # engine FIFO; GpSimd shares its SBUF port with DVE and
                    # just contends). DVE ~0.6ns/elem stays under the chunk
                    # DMA cadence.
                    tb = poolb.tile([P, 2, mb, N], cdt, tag="tb",
                                    name=f"tb_{c}")[:, :, :cmb]
                    nc.vector.tensor_copy(tb, tf)
                last_chunk = c == len(sched) - 1
                if cdt == mybir.dt.float8e4:
                    # DoubleRow: one matmul contracts 2 k-blocks (256 deep),
                    # 2 fp8 elements per PE cell. lhsT/rhs get 3D APs
                    # [K=128, 2, cols]; the (k, pair) -> d mapping just has
                    # to agree between the two operands, which it does since
                    # both tiles share the same [P, nb, N] layout.
                    assert cmb % 2 == 0, cmb
                    dr = mybir.MatmulPerfMode.DoubleRow
                    if not last_chunk:
                        for nb in range(0, cmb, 2):
                            gb = b0 + nb
                            for ib in range(2):
                                nc.tensor.matmul(
                                    acc[ib],
                                    tb[:, 0, nb:nb + 2, ib * P:(ib + 1) * P],
                                    tb[:, 1, nb:nb + 2, :],
                                    start=(gb == 0),
                                    stop=False,
                                    perf_mode=dr,
                                )
                        if c < fill_until:
                            _keep_warm(nfill)
                    else:
                        for ib in range(2):
                            for nb in range(0, cmb, 2):
                                nc.tensor.matmul(
                                    acc[ib],
                                    tb[:, 0, nb:nb + 2, ib * P:(ib + 1) * P],
                                    tb[:, 1, nb:nb + 2, :],
                                    start=False,
                                    stop=(nb == cmb - 2),
                                    perf_mode=dr,
                                )
                            o = outpool.tile([P, N], mybir.dt.float32,
                                             tag=f"o{ib}", name=f"o{ib}")
                            # different engines/rings per half so the two
                            # copies + stores drain in parallel
                            if ib == 0:
                                nc.vector.tensor_copy(o, acc[ib])
                                nc.sync.dma_start(
                                    out=out.ap()[ib * P:(ib + 1) * P, :],
                                    in_=o)
                            else:
                                nc.scalar.copy(o, acc[ib])
                                nc.scalar.dma_start(
                                    out=out.ap()[ib * P:(ib + 1) * P, :],
                                    in_=o)
                elif not last_chunk:
                    for nb in range(cmb):
                        gb = b0 + nb
                        for ib in range(2):
                            nc.tensor.matmul(
                                acc[ib],
                                tb[:, 0, nb, ib * P:(ib + 1) * P],  # lhsT
                                tb[:, 1, nb, :],                    # rhs
                                start=(gb == 0),
                                stop=False,
                            )
                else:
                    # ib-major in the last chunk: acc[0] finishes first so
                    # its PSUM copy + store overlap acc[1]'s final matmuls.
                    for ib in range(2):
                        for nb in range(cmb):
                            nc.tensor.matmul(
                                acc[ib],
                                tb[:, 0, nb, ib * P:(ib + 1) * P],
                                tb[:, 1, nb, :],
                                start=False,
                                stop=(nb == cmb - 1),
                            )
                        o = outpool.tile([P, N], mybir.dt.float32,
                                         tag=f"o{ib}", name=f"o{ib}")
                        nc.vector.tensor_copy(o, acc[ib])
                        nc.sync.dma_start(
                            out=out.ap()[ib * P:(ib + 1) * P, :], in_=o)
                b0 += cmb

    nc.compile()
    return nc


def _get_nc():
    if "nc" not in _built:
        # bufs_f=8: deep enough that the DMA rings never wait on buffer
        # release during the cold-PE fill phase. (A full 18-buffer
        # preload = 144 KB/partition dies on HW with
        # NRT_EXEC_UNIT_UNRECOVERABLE, though CoreSim accepts it.)
        # nfill=1: one dummy matmul (~0.25us) after each chunk's group
        # lifts PE duty from ~68% to ~78% of the DMA cadence so the HAM
        # clock gate stays at 2.4 GHz through the whole stream; it uses
        # a third of the per-chunk TensorE slack, so it never becomes
        # the pacer.
        _built["nc"] = _build(bufs_f=8, nfill=1, fill_until=10**9)
    return _built["nc"]


def _gram_partials(in_maps, trace=False):
    global _last_results
    from concourse.bass_utils import run_bass_kernel_spmd

    nc = _get_nc()
    res = run_bass_kernel_spmd(nc, in_maps, core_ids=list(range(NCORES)),
                               trace=trace)
    _last_results = res
    return [r["out"] for r in res.results]


def _pack_core(f1, f2, k, dloc=DLOC, mb=MB, stripe=1, dtype=None):
    """Pack core k's d-chunks of f1, f2 [N, D] into one [2*dloc, N] buffer.

    Chunk c covering blocks [b0, b0+cmb): the f1 tile occupies rows
    [2*b0*P, (2*b0+cmb)*P) and the f2 tile the next cmb*P rows, each in
    [P, cmb, N] SBUF tile order (row p*cmb+nb holds f[:, (b0+nb)*P+p]).

    dtype: on-wire dtype (default fp8e4). Cast happens before the
    transpose so the shuffle moves 1-byte elements.
    """
    if dtype is None:
        import ml_dtypes
        dtype = ml_dtypes.float8_e4m3
    x1 = f1[:, k * dloc:(k + 1) * dloc].astype(dtype)
    x2 = f2[:, k * dloc:(k + 1) * dloc].astype(dtype)
    sched, offs, sizes = _stripe_plan(dloc, mb, stripe)
    outs = [np.empty((sz, N), dtype=dtype) for sz in sizes]
    b0 = 0
    for c, cmb in enumerate(sched):
        tgt, r0 = offs[c]
        for j, x in enumerate((x1, x2)):
            sl = x[:, b0 * P:(b0 + cmb) * P]              # [N, cmb*P]
            t = sl.reshape(N, cmb, P).transpose(2, 1, 0)  # [P, cmb, N]
            rr = r0 + j * cmb * P
            outs[tgt][rr:rr + cmb * P] = t.reshape(cmb * P, N)
        b0 += cmb
    return {f"ft{s}": outs[s] for s in range(stripe)}


def kernel(V1, V2):
    V1 = np.asarray(V1, dtype=np.float32)
    V2 = np.asarray(V2, dtype=np.float32)
    f1 = V1.reshape(N, D)
    f2 = V2.reshape(N, D)

    from concurrent.futures import ThreadPoolExecutor
    with ThreadPoolExecutor(NCORES) as ex:
        in_maps = list(ex.map(lambda k: _pack_core(f1, f2, k), range(NCORES)))
    partials = _gram_partials(in_maps)

    Z = np.zeros((N, N), dtype=np.float64)
    for pmat in partials:
        Z += pmat
    Z /= D

    eps = 1e-12
    p = 1.0 / (1.0 + np.exp(-Z))
    p = np.clip(p, eps, 1.0 - eps)
    lab = np.eye(N, dtype=np.float64)
    loss = -np.mean(lab * np.log(p) + (1.0 - lab) * np.log1p(-p))
    return np.array(loss, dtype=np.float32)


def _selftest_sim():
    """Scaled-down correctness check in CoreSim (no hardware)."""
    import ml_dtypes
    from concourse.bass_interp import CoreSim

    dloc, mb = 2048, 4
    nc = _build(dloc=dloc, mb=mb)
    rng = np.random.default_rng(0)
    a = rng.standard_normal((N, dloc)).astype(np.float32)  # [N, dloc] like f1
    b = rng.standard_normal((N, dloc)).astype(np.float32)

    sim = CoreSim(nc)
    for name, arr in _pack_core(a, b, 0, dloc=dloc, mb=mb).items():
        sim.tensor(name)[:] = arr
    sim.simulate()
    got = np.array(sim.tensor("out"))
    # expected: exact product of the fp8-quantized inputs
    qa = a.astype(ml_dtypes.float8_e4m3).astype(np.float64)
    qb = b.astype(ml_dtypes.float8_e4m3).astype(np.float64)
    want = qa @ qb.T
    err = np.abs(got - want).max() / np.abs(want).max()
    print("selftest rel err vs quantized product:", err)
    assert err < 1e-2, err
    print("SELFTEST PASSED")


if __name__ == "__main__":
    _selftest_sim()



# revision 23
# speedup vs baseline: 1.0745x; 1.0048x over previous
"""CPC spatial BCE loss kernel for 8 TRN2 NeuronCores.

Computation: loss = BCE(sigmoid((V1.reshape(N,D) @ V2.reshape(N,D).T) / D), eye(N))
with N=256, D=64*64*64=262144.

Strategy (memory-regime): shard the contraction dim D across the 8 cores
and quantize to fp8e4 on the host, so each core streams 16 MB -- 1/4 of
the fp32 bytes and the minimum HBM traffic for an exact-shape Gram
computation (the 2e-2 loss tolerance dwarfs fp8 input rounding: measured
end-to-end effect ~1e-8 relative, since the loss is log(2) + O(1e-5) and
z-values are ~N(0, 1/sqrt(D))). Each core computes a partial [256, 256]
Gram over its 32768-wide slice of D. The host pre-tiles each core's
slice into the exact SBUF layout so every chunk DMA is one contiguous
read landing the contraction dim on partitions -- no on-device work but
the matmuls themselves.

Device pipeline per chunk: fp8 tiles stream on the two HWDGE rings (SP
ring f1 tile, ACT ring f2 tile; together ~410 GB/s, ~95% of the 435 GB/s
SBUF fabric ceiling); TensorE runs DoubleRow matmuls (2 fp8 values per
PE cell = 256-deep contraction per instruction) into 2 PSUM
accumulators, consuming each 1 MB chunk in ~1.75 us against the ~2.5 us
DMA cadence, so the stream stays DMA-paced. Dummy warm-up matmuls at
kernel start plus one per chunk keep the PE HAM clock gate at 2.4 GHz
(a cold 1.2 GHz PE otherwise outpaces nothing, backs up the SBUF pool,
and throttles the DMA rings mid-fill).

The partial Gram matrices are summed on the host (the unshard step for a
sum-sharded value) and the final sigmoid+BCE over 256x256 values is a
negligible epilogue done in numpy.
"""

import numpy as np

N = 256
D = 64 * 64 * 64  # 262144
NCORES = 8
DLOC = D // NCORES  # 32768
P = 128  # SBUF partitions
MB = 16  # max d-blocks of 128 per DMA chunk

_built = {}
_last_results = None  # test harness reads profiling info from here


def _sched(dloc=DLOC, mb=MB):
    """Chunk schedule in 128-d blocks: small head (fill) / tail (drain)."""
    nblocks = dloc // P
    if nblocks <= 4 * mb:
        sched = []
        rem = nblocks
        while rem > 0:
            s = min(4, rem)
            sched.append(s)
            rem -= s
        return sched
    # Head [8,8]: big enough for efficient DMA descriptor lines (2KB per
    # partition), small enough to start compute early. Body: mb-block
    # chunks. Tail [*, 2]: the drain after the last DMA byte is just one
    # 2-block chunk; per-chunk sem/restart overhead made a long small
    # tail a net loss.
    sched = [8, 8]
    while sum(sched) + mb <= nblocks - 16:
        sched.append(mb)
    rem = nblocks - sum(sched)
    if rem > 2:
        sched.append(rem - 2)
        sched.append(2)
    elif rem:
        sched.append(rem)
    assert sum(sched) == nblocks, (sched, nblocks)
    assert all(s % 2 == 0 for s in sched), sched
    return sched


def _stripe_plan(dloc, mb, stripe):
    """Assign chunk c -> (tensor c%stripe, row offset within it)."""
    sched = _sched(dloc, mb)
    offs, sizes = [], [0] * stripe
    for c, cmb in enumerate(sched):
        t = c % stripe
        offs.append((t, sizes[t]))
        sizes[t] += 2 * cmb * P
    return sched, offs, sizes


def _build(dloc=DLOC, mb=MB, compute_dtype="fp8", bufs=3, bufs_f=None,
           bufs_b=None, stripe=1, nwarm=10, nfill=0, fill_until=12):
    """Build + bacc-compile the per-core Bass kernel.

    Per-core inputs f1t, f2t: [dloc, N] fp32, host pre-tiled so chunk c
    (covering blocks [b0, b0+cmb) of 128 d-values) occupies rows
    [b0*P, (b0+cmb)*P) with row r = b0*P + p*cmb + nb holding
    f.T[d, :] for d = core_off + (b0+nb)*P + p  -- i.e. each chunk DMA
    is one contiguous read mapping partition p <- d within block.
    Output: out[i, j] = sum_d f1t[d, i] * f2t[d, j]   (partial Gram)
    """
    import concourse.mybir as mybir
    from concourse import bacc
    from concourse.bass import MemorySpace
    from concourse.tile import TileContext

    nblocks = dloc // P

    # fp8 mode: inputs are cast to fp8e4 on the host, so HBM traffic is
    # 1/4 of fp32 and there is no on-device cast at all. TensorE runs
    # DoubleRow matmuls (2 k-blocks of 128 per instruction, 2 fp8 values
    # per PE cell) to stay under the DMA cadence.
    if compute_dtype == "fp8":
        idt = cdt = mybir.dt.float8e4
    elif compute_dtype == "bf16":
        idt, cdt = mybir.dt.float32, mybir.dt.bfloat16
    else:
        idt = cdt = mybir.dt.float32

    nc = bacc.Bacc("TRN2", target_bir_lowering=False, debug=False,
                   num_devices=NCORES)
    # Both tensors packed chunk-interleaved [f1_c | f2_c | f1_c+1 ...] so the
    # two concurrent ring reads hit adjacent address regions (uniform HBM
    # channel striping instead of two far-apart colliding streams). With
    # stripe > 1, chunks round-robin over separate DRAM allocations.
    sched, offs, sizes = _stripe_plan(dloc, mb, stripe)
    fts = [nc.dram_tensor(f"ft{s}", (sizes[s], N), idt,
                          kind="ExternalInput")
           for s in range(stripe)]
    out = nc.dram_tensor("out", (N, N), mybir.dt.float32,
                         kind="ExternalOutput")

    fvs = [t.ap() for t in fts]

    with TileContext(nc) as tc:
        with tc.tile_pool(name="psum", bufs=1, space=MemorySpace.PSUM) as psum_pool, \
             tc.tile_pool(name="sbuff", bufs=bufs_f or bufs + 1) as poolf, \
             tc.tile_pool(name="sbufb", bufs=bufs_b or bufs) as poolb, \
             tc.tile_pool(name="outp", bufs=1) as outpool:
            acc = [psum_pool.tile([P, N], mybir.dt.float32, tag=f"acc{ib}",
                                  name=f"acc{ib}")
                   for ib in range(2)]
            if nwarm:
                # PE warm-up: the HAM clock gate keeps the PE at 1.2 GHz
                # until it sees ~3.4us of sustained activity. Without this,
                # the first ~14us of real matmuls run at half clock, the
                # cold PE outpaces nothing and the DMA rings throttle on
                # full SBUF buffers. These dummy matmuls run during the
                # first chunk DMAs (they depend only on the memset) and
                # flip the gate before real data lands.
                wsrc = outpool.tile([P, 512], cdt, tag="wsrc", name="wsrc")
                wacc = psum_pool.tile([P, 512], mybir.dt.float32,
                                      tag="wacc", name="wacc")
                nc.any.memset(wsrc, 0)
                for i in range(nwarm):
                    nc.tensor.matmul(wacc, wsrc[:, :P], wsrc,
                                     start=True, stop=True)

                def _keep_warm(n):
                    # HAM watches PE duty cycle per free-running ~3.4us
                    # window; the DMA-paced gaps between chunk bursts
                    # during pipeline fill re-throttle the clock (and a
                    # cold PE then paces the pipeline). Dummy matmuls
                    # absorb the idle gap.
                    for _ in range(n):
                        nc.tensor.matmul(wacc, wsrc[:, :P], wsrc,
                                         start=True, stop=True)
            else:
                def _keep_warm(n):
                    pass
            b0 = 0
            for c, cmb in enumerate(sched):
                # f1 tile and f2 tile are adjacent in the packed buffer;
                # the two HWDGE rings (SP / ACT) stream them concurrently.
                tgt, r0 = offs[c]
                fv = fvs[tgt]
                r1 = fv[r0:r0 + cmb * P]
                r2 = fv[r0 + cmb * P:r0 + 2 * cmb * P]
                tf = poolf.tile([P, 2, mb, N], idt, tag="tf",
                                name=f"tf_{c}")[:, :, :cmb]
                nc.sync.dma_start(
                    out=tf[:, 0], in_=r1.rearrange("(p nb) i -> p nb i", p=P))
                nc.scalar.dma_start(
                    out=tf[:, 1], in_=r2.rearrange("(p nb) i -> p nb i", p=P))
                if cdt == idt:
                    tb = tf
                else:
                    # cast fp32->bf16 on DVE only: SP/ACT stay pure DMA rings
                    # (casts on ACT block its ring's next DMA issue in the
                    # engine FIFO; GpSimd shares its SBUF port with DVE and
                    # just contends). DVE ~0.6ns/elem stays under the chunk
                    # DMA cadence.
                    tb = poolb.tile([P, 2, mb, N], cdt, tag="tb",
                                    name=f"tb_{c}")[:, :, :cmb]
                    nc.vector.tensor_copy(tb, tf)
                last_chunk = c == len(sched) - 1
                if cdt == mybir.dt.float8e4:
                    # DoubleRow: one matmul contracts 2 k-blocks (256 deep),
                    # 2 fp8 elements per PE cell. lhsT/rhs get 3D APs
                    # [K=128, 2, cols]; the (k, pair) -> d mapping just has
                    # to agree between the two operands, which it does since
                    # both tiles share the same [P, nb, N] layout.
                    assert cmb % 2 == 0, cmb
                    dr = mybir.MatmulPerfMode.DoubleRow
                    if not last_chunk:
                        for nb in range(0, cmb, 2):
                            gb = b0 + nb
                            for ib in range(2):
                                nc.tensor.matmul(
                                    acc[ib],
                                    tb[:, 0, nb:nb + 2, ib * P:(ib + 1) * P],
                                    tb[:, 1, nb:nb + 2, :],
                                    start=(gb == 0),
                                    stop=False,
                                    perf_mode=dr,
                                )
                        if c < fill_until:
                            _keep_warm(nfill)
                    else:
                        for ib in range(2):
                            for nb in range(0, cmb, 2):
                                nc.tensor.matmul(
                                    acc[ib],
                                    tb[:, 0, nb:nb + 2, ib * P:(ib + 1) * P],
                                    tb[:, 1, nb:nb + 2, :],
                                    start=False,
                                    stop=(nb == cmb - 2),
                                    perf_mode=dr,
                                )
                            o = outpool.tile([P, N], mybir.dt.float32,
                                             tag=f"o{ib}", name=f"o{ib}")
                            # different engines/rings per half so the two
                            # copies + stores drain in parallel
                            if ib == 0:
                                nc.vector.tensor_copy(o, acc[ib])
                                nc.sync.dma_start(
                                    out=out.ap()[ib * P:(ib + 1) * P, :],
                                    in_=o)
                            else:
                                nc.scalar.copy(o, acc[ib])
                                nc.scalar.dma_start(
                                    out=out.ap()[ib * P:(ib + 1) * P, :],
                                    in_=o)
                elif not last_chunk:
                    for nb in range(cmb):
                        gb = b0 + nb
                        for ib in range(2):
                            nc.tensor.matmul(
                                acc[ib],
                                tb[:, 0, nb, ib * P:(ib + 1) * P],  # lhsT
                                tb[:, 1, nb, :],                    # rhs
                                start=(gb == 0),
                                stop=False,
                            )
                else:
                    # ib-major in the last chunk: acc[0] finishes first so
                    # its PSUM copy + store overlap acc[1]'s final matmuls.
                    for ib in range(2):
                        for nb in range(cmb):
                            nc.tensor.matmul(
                                acc[ib],
                                tb[:, 0, nb, ib * P:(ib + 1) * P],
                                tb[:, 1, nb, :],
                                start=False,
                                stop=(nb == cmb - 1),
                            )
                        o = outpool.tile([P, N], mybir.dt.float32,
                                         tag=f"o{ib}", name=f"o{ib}")
                        nc.vector.tensor_copy(o, acc[ib])
                        nc.sync.dma_start(
                            out=out.ap()[ib * P:(ib + 1) * P, :], in_=o)
                b0 += cmb

    nc.compile()
    return nc


def _get_nc():
    if "nc" not in _built:
        # bufs_f=8: deep enough that the DMA rings never wait on buffer
        # release during the cold-PE fill phase. (A full 18-buffer
        # preload = 144 KB/partition dies on HW with
        # NRT_EXEC_UNIT_UNRECOVERABLE, though CoreSim accepts it.)
        # nfill=1: one dummy matmul (~0.25us) after each chunk's group
        # lifts PE duty from ~68% to ~78% of the DMA cadence so the HAM
        # clock gate stays at 2.4 GHz through the whole stream; it uses
        # a third of the per-chunk TensorE slack, so it never becomes
        # the pacer.
        _built["nc"] = _build(bufs_f=8, nfill=2, fill_until=10**9)
    return _built["nc"]


def _gram_partials(in_maps, trace=False):
    global _last_results
    from concourse.bass_utils import run_bass_kernel_spmd

    nc = _get_nc()
    res = run_bass_kernel_spmd(nc, in_maps, core_ids=list(range(NCORES)),
                               trace=trace)
    _last_results = res
    return [r["out"] for r in res.results]


def _pack_core(f1, f2, k, dloc=DLOC, mb=MB, stripe=1, dtype=None):
    """Pack core k's d-chunks of f1, f2 [N, D] into one [2*dloc, N] buffer.

    Chunk c covering blocks [b0, b0+cmb): the f1 tile occupies rows
    [2*b0*P, (2*b0+cmb)*P) and the f2 tile the next cmb*P rows, each in
    [P, cmb, N] SBUF tile order (row p*cmb+nb holds f[:, (b0+nb)*P+p]).

    dtype: on-wire dtype (default fp8e4). Cast happens before the
    transpose so the shuffle moves 1-byte elements.
    """
    if dtype is None:
        import ml_dtypes
        dtype = ml_dtypes.float8_e4m3
    x1 = f1[:, k * dloc:(k + 1) * dloc].astype(dtype)
    x2 = f2[:, k * dloc:(k + 1) * dloc].astype(dtype)
    sched, offs, sizes = _stripe_plan(dloc, mb, stripe)
    outs = [np.empty((sz, N), dtype=dtype) for sz in sizes]
    b0 = 0
    for c, cmb in enumerate(sched):
        tgt, r0 = offs[c]
        for j, x in enumerate((x1, x2)):
            sl = x[:, b0 * P:(b0 + cmb) * P]              # [N, cmb*P]
            t = sl.reshape(N, cmb, P).transpose(2, 1, 0)  # [P, cmb, N]
            rr = r0 + j * cmb * P
            outs[tgt][rr:rr + cmb * P] = t.reshape(cmb * P, N)
        b0 += cmb
    return {f"ft{s}": outs[s] for s in range(stripe)}


def kernel(V1, V2):
    V1 = np.asarray(V1, dtype=np.float32)
    V2 = np.asarray(V2, dtype=np.float32)
    f1 = V1.reshape(N, D)
    f2 = V2.reshape(N, D)

    from concurrent.futures import ThreadPoolExecutor
    with ThreadPoolExecutor(NCORES) as ex:
        in_maps = list(ex.map(lambda k: _pack_core(f1, f2, k), range(NCORES)))
    partials = _gram_partials(in_maps)

    Z = np.zeros((N, N), dtype=np.float64)
    for pmat in partials:
        Z += pmat
    Z /= D

    eps = 1e-12
    p = 1.0 / (1.0 + np.exp(-Z))
    p = np.clip(p, eps, 1.0 - eps)
    lab = np.eye(N, dtype=np.float64)
    loss = -np.mean(lab * np.log(p) + (1.0 - lab) * np.log1p(-p))
    return np.array(loss, dtype=np.float32)


def _selftest_sim():
    """Scaled-down correctness check in CoreSim (no hardware)."""
    import ml_dtypes
    from concourse.bass_interp import CoreSim

    dloc, mb = 2048, 4
    nc = _build(dloc=dloc, mb=mb)
    rng = np.random.default_rng(0)
    a = rng.standard_normal((N, dloc)).astype(np.float32)  # [N, dloc] like f1
    b = rng.standard_normal((N, dloc)).astype(np.float32)

    sim = CoreSim(nc)
    for name, arr in _pack_core(a, b, 0, dloc=dloc, mb=mb).items():
        sim.tensor(name)[:] = arr
    sim.simulate()
    got = np.array(sim.tensor("out"))
    # expected: exact product of the fp8-quantized inputs
    qa = a.astype(ml_dtypes.float8_e4m3).astype(np.float64)
    qb = b.astype(ml_dtypes.float8_e4m3).astype(np.float64)
    want = qa @ qb.T
    err = np.abs(got - want).max() / np.abs(want).max()
    print("selftest rel err vs quantized product:", err)
    assert err < 1e-2, err
    print("SELFTEST PASSED")


if __name__ == "__main__":
    _selftest_sim()

